# revision 2
# baseline (speedup 1.0000x reference)
"""MoE expert-parallel kernel for Trainium2 (8 NeuronCores).

Strategy:
  - Host: route tokens to experts (stable sort by dispatch_order), pad each
    expert's token group to a common capacity PADC, build transposed activation
    buffers, cast weights/activations to bf16.
  - Device (SPMD, 8 cores, expert parallelism: 8 experts/core):
    per expert e: HT = gelu(w1[e]^T-tiled @ XT + b1)  (computed transposed,
    [F, tokens]), then Y = HT^T @ w2[e] + b2  ([tokens, D]) via PE matmuls
    with fp32 PSUM accumulation.
  - Host: scatter per-expert outputs back to original token order.

No cross-core collectives are needed: each core owns a disjoint set of
experts, hence a disjoint set of token rows of the output.
"""

import os
import sys

import numpy as np
import ml_dtypes

for _p in ("/opt/trn_rl_repo",):
    if _p not in sys.path:
        sys.path.insert(0, _p)

_BF16 = ml_dtypes.bfloat16

NUM_EXPERTS = 64
N_CORES = 8
E_LOCAL = NUM_EXPERTS // N_CORES  # 8 experts per core
D = 512
F = 2048
KD = D // 128   # 4 contraction tiles for layer 1
KF = F // 128   # 16 contraction tiles for layer 2

_nc_cache = {}


def _build_nc(PADC):
    """Build + compile the SPMD Bass program for per-expert capacity PADC."""
    import concourse.bacc as bacc
    import concourse.bass as bass
    import concourse.mybir as mybir
    import concourse.tile as tile

    fp32 = mybir.dt.float32
    bf16 = mybir.dt.bfloat16

    NT = PADC // 128  # token tiles (partition dim) per expert for layer 2
    # layer-1 token chunks (free dim of PSUM tile <= 512 fp32)
    chunks = []
    c0 = 0
    while c0 < PADC:
        cs = min(512, PADC - c0)
        chunks.append((c0, cs))
        c0 += cs

    nc = bacc.Bacc("TRN2", target_bir_lowering=False, debug=False)

    xt_d = nc.dram_tensor("xt", [KD, 128, E_LOCAL * PADC], bf16, kind="ExternalInput")
    w1_d = nc.dram_tensor("w1", [E_LOCAL, KD, 128, F], bf16, kind="ExternalInput")
    w2_d = nc.dram_tensor("w2", [E_LOCAL, KF, 128, D], bf16, kind="ExternalInput")
    b1_d = nc.dram_tensor("b1", [E_LOCAL, 128, KF], fp32, kind="ExternalInput")
    b2_d = nc.dram_tensor("b2", [E_LOCAL, D], fp32, kind="ExternalInput")
    y_d = nc.dram_tensor("y", [E_LOCAL * PADC, D], fp32, kind="ExternalOutput")

    with tile.TileContext(nc) as tc:
        with (
            tc.tile_pool(name="wpool", bufs=2) as wp,
            tc.tile_pool(name="xpool", bufs=2) as xp,
            tc.tile_pool(name="hpool", bufs=2) as hp,
            tc.tile_pool(name="ypool", bufs=4) as yp,
            tc.tile_pool(name="bias", bufs=1) as bp,
            tc.tile_pool(name="psh", bufs=4, space="PSUM") as psh,
            tc.tile_pool(name="psy", bufs=3, space="PSUM") as psy,
        ):
            # biases: loaded once for all local experts
            b1_sb = bp.tile([128, E_LOCAL, KF], fp32)
            nc.sync.dma_start(out=b1_sb[:], in_=b1_d[:].rearrange("e p f -> p e f"))
            # b2 broadcast across all 128 partitions (partition step 0)
            b2_sb = bp.tile([128, E_LOCAL, D], fp32)
            b2_ap = b2_d[:]
            b2_bc = bass.AP(
                tensor=b2_ap.tensor,
                offset=b2_ap.offset,
                ap=[[0, 128]] + [list(a) for a in b2_ap.ap],
            )
            nc.sync.dma_start(out=b2_sb[:], in_=b2_bc)

            for e in range(E_LOCAL):
                w1_sb = wp.tile([128, KD, F], bf16, tag="w1")
                nc.sync.dma_start(
                    out=w1_sb[:], in_=w1_d[e].rearrange("k p f -> p k f")
                )
                w2_sb = wp.tile([128, KF, D], bf16, tag="w2")
                nc.sync.dma_start(
                    out=w2_sb[:], in_=w2_d[e].rearrange("k p f -> p k f")
                )
                xt_sb = xp.tile([128, KD, PADC], bf16, tag="xt")
                nc.sync.dma_start(
                    out=xt_sb[:],
                    in_=xt_d[:, :, e * PADC:(e + 1) * PADC].rearrange(
                        "k p t -> p k t"
                    ),
                )

                # layer 1: HT[f*128:(f+1)*128, tok] = gelu(w1_tile.T @ XT + b1)
                ht_sb = hp.tile([128, KF, PADC], bf16, tag="ht")
                for f in range(KF):
                    for (c0, cs) in chunks:
                        ph = psh.tile([128, 512], fp32, tag="ph")
                        for k in range(KD):
                            nc.tensor.matmul(
                                ph[:, :cs],
                                lhsT=w1_sb[:, k, f * 128:(f + 1) * 128],
                                rhs=xt_sb[:, k, c0:c0 + cs],
                                start=(k == 0),
                                stop=(k == KD - 1),
                            )
                        nc.scalar.activation(
                            out=ht_sb[:, f, c0:c0 + cs],
                            in_=ph[:, :cs],
                            func=mybir.ActivationFunctionType.Gelu,
                            bias=b1_sb[:, e, f:f + 1],
                            scale=1.0,
                        )

                # layer 2: Y[t*128:(t+1)*128, :] = HT_tile.T @ w2 + b2
                for t in range(NT):
                    py = psy.tile([128, D], fp32, tag="py")
                    for k in range(KF):
                        nc.tensor.matmul(
                            py[:],
                            lhsT=ht_sb[:, k, t * 128:(t + 1) * 128],
                            rhs=w2_sb[:, k, :],
                            start=(k == 0),
                            stop=(k == KF - 1),
                        )
                    y_sb = yp.tile([128, D], fp32, tag="ysb")
                    nc.vector.tensor_add(y_sb[:], py[:], b2_sb[:, e, :])
                    nc.sync.dma_start(
                        out=y_d[e * PADC + t * 128: e * PADC + (t + 1) * 128, :],
                        in_=y_sb[:],
                    )

    nc.compile()
    return nc


def _get_nc(PADC):
    if PADC not in _nc_cache:
        _nc_cache[PADC] = _build_nc(PADC)
    return _nc_cache[PADC]


def kernel(**inputs):
    x = np.asarray(inputs["inputs"], dtype=np.float32)
    disp = np.asarray(inputs["dispatch_order"])
    w1 = np.asarray(inputs["w1"], dtype=np.float32)
    b1 = np.asarray(inputs["b1"], dtype=np.float32)
    w2 = np.asarray(inputs["w2"], dtype=np.float32)
    b2 = np.asarray(inputs["b2"], dtype=np.float32)

    B, S, Dd = x.shape
    assert Dd == D
    T = B * S
    xf = x.reshape(T, D)
    e = disp.astype(np.int64)

    counts = np.bincount(e, minlength=NUM_EXPERTS)
    PADC = int(-(-int(counts.max()) // 128) * 128)  # round up to multiple of 128
    order = np.argsort(e, kind="stable")
    xs = xf[order]  # tokens grouped by expert, original order within expert
    offs = np.zeros(NUM_EXPERTS + 1, dtype=np.int64)
    np.cumsum(counts, out=offs[1:])

    # weights in device layout
    w1b = w1.astype(_BF16).reshape(NUM_EXPERTS, KD, 128, F)
    w2b = w2.astype(_BF16).reshape(NUM_EXPERTS, KF, 128, D)
    b1r = np.ascontiguousarray(
        b1.reshape(NUM_EXPERTS, KF, 128).transpose(0, 2, 1)
    )  # [E, 128, KF]

    xsb = xs.astype(_BF16)

    in_maps = []
    for c in range(N_CORES):
        xt = np.zeros((KD, 128, E_LOCAL * PADC), dtype=_BF16)
        for j in range(E_LOCAL):
            ei = c * E_LOCAL + j
            cnt = int(counts[ei])
            if cnt:
                xe = xsb[offs[ei]:offs[ei + 1]]  # [cnt, D]
                xt[:, :, j * PADC:j * PADC + cnt] = xe.T.reshape(KD, 128, cnt)
        sl = slice(c * E_LOCAL, (c + 1) * E_LOCAL)
        in_maps.append({
            "xt": xt,
            "w1": np.ascontiguousarray(w1b[sl]),
            "w2": np.ascontiguousarray(w2b[sl]),
            "b1": np.ascontiguousarray(b1r[sl]),
            "b2": np.ascontiguousarray(b2[sl]),
        })

    nc = _get_nc(PADC)
    global _last_in_maps
    _last_in_maps = in_maps
    from concourse.bass_utils import run_bass_kernel_spmd

    res = run_bass_kernel_spmd(nc, in_maps, core_ids=list(range(N_CORES)))

    out_sorted = np.empty((T, D), dtype=np.float32)
    for c in range(N_CORES):
        y = res.results[c]["y"]
        for j in range(E_LOCAL):
            ei = c * E_LOCAL + j
            cnt = int(counts[ei])
            if cnt:
                out_sorted[offs[ei]:offs[ei + 1]] = y[j * PADC:j * PADC + cnt]

    out = np.empty((T, D), dtype=np.float32)
    out[order] = out_sorted
    return out.reshape(B, S, D)


# revision 3
# speedup vs baseline: 1.1354x; 1.1354x over previous
"""MoE expert-parallel kernel for Trainium2 (8 NeuronCores).

Strategy:
  - Host: route tokens to experts (stable sort by dispatch_order). Experts are
    assigned to (core, slot) pairs by descending token count: slot j of core c
    gets the (8*j + c)-th most-loaded expert, so all cores see nearly identical
    work and slot j's capacity cap_j = max over cores of its count (tight).
  - Device (SPMD, 8 cores, 8 expert slots/core):
    per slot: HT = gelu(w1^T-tiled @ XT + b1) computed transposed [F, tokens],
    then Y = HT^T @ w2 + b2 [tokens, D]; bf16 operands, fp32 PSUM accumulation.
  - Host: scatter per-expert outputs back to original token order.

No cross-core collectives: each core owns a disjoint set of experts, hence a
disjoint set of output token rows.
"""

import sys

import numpy as np
import ml_dtypes

for _p in ("/opt/trn_rl_repo",):
    if _p not in sys.path:
        sys.path.insert(0, _p)

_BF16 = ml_dtypes.bfloat16

NUM_EXPERTS = 64
N_CORES = 8
E_LOCAL = NUM_EXPERTS // N_CORES  # 8 expert slots per core
D = 512
F = 2048
KD = D // 128   # 4 contraction tiles for layer 1
KF = F // 128   # 16 contraction tiles for layer 2

_nc_cache = {}


def _slot_geometry(caps):
    """Per-slot column offsets for xt and row offsets for y."""
    xoff = [0]
    yoff = [0]
    for c in caps:
        xoff.append(xoff[-1] + c)
        yoff.append(yoff[-1] + (-(-c // 128)) * 128)
    return xoff, yoff


def _build_nc(caps):
    """Build + compile the SPMD Bass program for per-slot capacities `caps`."""
    import concourse.bacc as bacc
    import concourse.bass as bass
    import concourse.mybir as mybir
    import concourse.tile as tile

    fp32 = mybir.dt.float32
    bf16 = mybir.dt.bfloat16

    xoff, yoff = _slot_geometry(caps)
    XCOLS = xoff[-1]
    YROWS = yoff[-1]
    CAPMAX = max(caps)

    nc = bacc.Bacc("TRN2", target_bir_lowering=False, debug=False)

    xt_d = nc.dram_tensor("xt", [KD, 128, XCOLS], bf16, kind="ExternalInput")
    w1_d = nc.dram_tensor("w1", [E_LOCAL, KD, 128, F], bf16, kind="ExternalInput")
    w2_d = nc.dram_tensor("w2", [E_LOCAL, KF, 128, D], bf16, kind="ExternalInput")
    b1_d = nc.dram_tensor("b1", [E_LOCAL, 128, KF], fp32, kind="ExternalInput")
    b2_d = nc.dram_tensor("b2", [E_LOCAL, D], fp32, kind="ExternalInput")
    y_d = nc.dram_tensor("y", [YROWS, D], fp32, kind="ExternalOutput")

    with tile.TileContext(nc) as tc:
        with (
            tc.tile_pool(name="wpool", bufs=2) as wp,
            tc.tile_pool(name="xpool", bufs=2) as xp,
            tc.tile_pool(name="hpool", bufs=2) as hp,
            tc.tile_pool(name="ypool", bufs=4) as yp,
            tc.tile_pool(name="bias", bufs=1) as bp,
            tc.tile_pool(name="psh", bufs=4, space="PSUM") as psh,
            tc.tile_pool(name="psy", bufs=3, space="PSUM") as psy,
        ):
            w1_sbs = [None] * E_LOCAL
            w2_sbs = [None] * E_LOCAL
            xt_sbs = [None] * E_LOCAL

            def load_slot(e, split_w1):
                xt_sb = xp.tile([128, KD, CAPMAX], bf16, tag="xt")
                nc.sync.dma_start(
                    out=xt_sb[:, :, :caps[e]],
                    in_=xt_d[:, :, xoff[e]:xoff[e + 1]].rearrange("k p t -> p k t"),
                )
                w1_sb = wp.tile([128, KD, F], bf16, tag="w1")
                if split_w1:
                    # chunk by F so the first layer-1 matmuls start after ~1MB
                    for f0 in range(0, F, 512):
                        nc.sync.dma_start(
                            out=w1_sb[:, :, f0:f0 + 512],
                            in_=w1_d[e, :, :, f0:f0 + 512].rearrange(
                                "k p f -> p k f"
                            ),
                        )
                else:
                    nc.sync.dma_start(
                        out=w1_sb[:], in_=w1_d[e].rearrange("k p f -> p k f")
                    )
                w2_sb = wp.tile([128, KF, D], bf16, tag="w2")
                nc.sync.dma_start(
                    out=w2_sb[:], in_=w2_d[e].rearrange("k p f -> p k f")
                )
                xt_sbs[e], w1_sbs[e], w2_sbs[e] = xt_sb, w1_sb, w2_sb

            # slot 0 inputs issued first so PE can start ASAP
            load_slot(0, split_w1=True)

            # biases (small / off critical path; b2 broadcast on gpsimd queue)
            b1_sb = bp.tile([128, E_LOCAL, KF], fp32)
            nc.gpsimd.dma_start(out=b1_sb[:], in_=b1_d[:].rearrange("e p f -> p e f"))
            b2_sb = bp.tile([128, E_LOCAL, D], fp32)
            b2_ap = b2_d[:]
            b2_bc = bass.AP(
                tensor=b2_ap.tensor,
                offset=b2_ap.offset,
                ap=[[0, 128]] + [list(a) for a in b2_ap.ap],
            )
            nc.gpsimd.dma_start(out=b2_sb[:], in_=b2_bc)

            for e in range(E_LOCAL):
                cap = caps[e]
                if e + 1 < E_LOCAL:
                    load_slot(e + 1, split_w1=False)
                w1_sb, w2_sb, xt_sb = w1_sbs[e], w2_sbs[e], xt_sbs[e]

                # layer-1 token chunks (PSUM free dim <= 512 fp32)
                chunks = []
                c0 = 0
                while c0 < cap:
                    cs = min(512, cap - c0)
                    chunks.append((c0, cs))
                    c0 += cs

                # layer 1: HT[f-tile, tok] = gelu(w1_tile.T @ XT + b1)
                ht_sb = hp.tile([128, KF, CAPMAX], bf16, tag="ht")
                for f in range(KF):
                    for (c0, cs) in chunks:
                        ph = psh.tile([128, 512], fp32, tag="ph")
                        for k in range(KD):
                            nc.tensor.matmul(
                                ph[:, :cs],
                                lhsT=w1_sb[:, k, f * 128:(f + 1) * 128],
                                rhs=xt_sb[:, k, c0:c0 + cs],
                                start=(k == 0),
                                stop=(k == KD - 1),
                            )
                        nc.scalar.activation(
                            out=ht_sb[:, f, c0:c0 + cs],
                            in_=ph[:, :cs],
                            func=mybir.ActivationFunctionType.Gelu,
                            bias=b1_sb[:, e, f:f + 1],
                            scale=1.0,
                        )

                # layer 2: Y[t-tile, :] = HT_tile.T @ w2 + b2
                NT = -(-cap // 128)
                for t in range(NT):
                    tt = min(128, cap - t * 128)
                    py = psy.tile([128, D], fp32, tag="py")
                    for k in range(KF):
                        nc.tensor.matmul(
                            py[:tt, :],
                            lhsT=ht_sb[:, k, t * 128:t * 128 + tt],
                            rhs=w2_sb[:, k, :],
                            start=(k == 0),
                            stop=(k == KF - 1),
                        )
                    y_sb = yp.tile([128, D], fp32, tag="ysb")
                    nc.vector.tensor_add(y_sb[:tt, :], py[:tt, :], b2_sb[:tt, e, :])
                    nc.sync.dma_start(
                        out=y_d[yoff[e] + t * 128: yoff[e] + t * 128 + tt, :],
                        in_=y_sb[:tt, :],
                    )

    nc.compile()
    return nc


def _get_nc(caps):
    key = tuple(caps)
    if key not in _nc_cache:
        _nc_cache[key] = _build_nc(key)
    return _nc_cache[key]


def kernel(**inputs):
    x = np.asarray(inputs["inputs"], dtype=np.float32)
    disp = np.asarray(inputs["dispatch_order"])
    w1 = np.asarray(inputs["w1"], dtype=np.float32)
    b1 = np.asarray(inputs["b1"], dtype=np.float32)
    w2 = np.asarray(inputs["w2"], dtype=np.float32)
    b2 = np.asarray(inputs["b2"], dtype=np.float32)

    B, S, Dd = x.shape
    assert Dd == D
    T = B * S
    xf = x.reshape(T, D)
    e = disp.astype(np.int64)

    counts = np.bincount(e, minlength=NUM_EXPERTS)
    order = np.argsort(e, kind="stable")
    xs = xf[order]  # tokens grouped by expert, original order within expert
    offs = np.zeros(NUM_EXPERTS + 1, dtype=np.int64)
    np.cumsum(counts, out=offs[1:])

    # assign experts to (slot, core): slot j of core c gets the (8j+c)-th
    # most-loaded expert -> tight per-slot caps, balanced cores
    by_load = np.argsort(-counts, kind="stable")
    slot_expert = by_load.reshape(E_LOCAL, N_CORES)  # [slot, core] -> expert id
    caps = tuple(int(counts[slot_expert[j]].max()) for j in range(E_LOCAL))
    xoff, yoff = _slot_geometry(caps)

    # weights in device layout
    w1b = w1.astype(_BF16).reshape(NUM_EXPERTS, KD, 128, F)
    w2b = w2.astype(_BF16).reshape(NUM_EXPERTS, KF, 128, D)
    b1r = np.ascontiguousarray(
        b1.reshape(NUM_EXPERTS, KF, 128).transpose(0, 2, 1)
    )  # [E, 128, KF]
    xsb = xs.astype(_BF16)

    in_maps = []
    for c in range(N_CORES):
        eids = [int(slot_expert[j, c]) for j in range(E_LOCAL)]
        xt = np.zeros((KD, 128, xoff[-1]), dtype=_BF16)
        for j, ei in enumerate(eids):
            cnt = int(counts[ei])
            if cnt:
                xe = xsb[offs[ei]:offs[ei + 1]]  # [cnt, D]
                xt[:, :, xoff[j]:xoff[j] + cnt] = xe.T.reshape(KD, 128, cnt)
        in_maps.append({
            "xt": xt,
            "w1": np.ascontiguousarray(w1b[eids]),
            "w2": np.ascontiguousarray(w2b[eids]),
            "b1": np.ascontiguousarray(b1r[eids]),
            "b2": np.ascontiguousarray(b2[eids]),
        })

    nc = _get_nc(caps)
    global _last_in_maps
    _last_in_maps = in_maps
    from concourse.bass_utils import run_bass_kernel_spmd

    res = run_bass_kernel_spmd(nc, in_maps, core_ids=list(range(N_CORES)))

    out_sorted = np.empty((T, D), dtype=np.float32)
    for c in range(N_CORES):
        y = res.results[c]["y"]
        for j in range(E_LOCAL):
            ei = int(slot_expert[j, c])
            cnt = int(counts[ei])
            if cnt:
                out_sorted[offs[ei]:offs[ei + 1]] = y[yoff[j]:yoff[j] + cnt]

    out = np.empty((T, D), dtype=np.float32)
    out[order] = out_sorted
    return out.reshape(B, S, D)


# revision 5
# speedup vs baseline: 1.1469x; 1.0102x over previous
"""MoE expert-parallel kernel for Trainium2 (8 NeuronCores).

Strategy:
  - Host: route tokens to experts (stable sort by dispatch_order). Experts are
    assigned to (core, slot) pairs by descending token count: slot j of core c
    gets the (8*j + c)-th most-loaded expert, so all cores see nearly identical
    work and slot j's capacity cap_j = max over cores of its count (tight).
  - Device (SPMD, 8 cores, 8 expert slots/core):
    per slot: HT = gelu(w1^T-tiled @ XT + b1) computed transposed [F, tokens],
    then Y = HT^T @ w2 + b2 [tokens, D]; bf16 operands, fp32 PSUM accumulation.
  - Host: scatter per-expert outputs back to original token order.

No cross-core collectives: each core owns a disjoint set of experts, hence a
disjoint set of output token rows.
"""

import sys

import numpy as np
import ml_dtypes

for _p in ("/opt/trn_rl_repo",):
    if _p not in sys.path:
        sys.path.insert(0, _p)

_BF16 = ml_dtypes.bfloat16

NUM_EXPERTS = 64
N_CORES = 8
E_LOCAL = NUM_EXPERTS // N_CORES  # 8 expert slots per core
D = 512
F = 2048
KD = D // 128   # 4 contraction tiles for layer 1
KF = F // 128   # 16 contraction tiles for layer 2

_nc_cache = {}


def _slot_geometry(caps):
    """Per-slot column offsets for xt and row offsets for y."""
    xoff = [0]
    yoff = [0]
    for c in caps:
        xoff.append(xoff[-1] + c)
        yoff.append(yoff[-1] + (-(-c // 128)) * 128)
    return xoff, yoff


def _build_nc(caps):
    """Build + compile the SPMD Bass program for per-slot capacities `caps`."""
    import concourse.bacc as bacc
    import concourse.bass as bass
    import concourse.mybir as mybir
    import concourse.tile as tile

    fp32 = mybir.dt.float32
    bf16 = mybir.dt.bfloat16

    xoff, yoff = _slot_geometry(caps)
    XCOLS = xoff[-1]
    YROWS = yoff[-1]
    CAPMAX = max(caps)

    nc = bacc.Bacc("TRN2", target_bir_lowering=False, debug=False)

    xt_d = nc.dram_tensor("xt", [KD, 128, XCOLS], bf16, kind="ExternalInput")
    w1_d = nc.dram_tensor("w1", [E_LOCAL, KD, 128, F], bf16, kind="ExternalInput")
    w2_d = nc.dram_tensor("w2", [E_LOCAL, KF, 128, D], bf16, kind="ExternalInput")
    b1_d = nc.dram_tensor("b1", [E_LOCAL, 128, KF], fp32, kind="ExternalInput")
    b2_d = nc.dram_tensor("b2", [E_LOCAL, D], fp32, kind="ExternalInput")
    y_d = nc.dram_tensor("y", [YROWS, D], fp32, kind="ExternalOutput")

    with tile.TileContext(nc) as tc:
        with (
            tc.tile_pool(name="wpool", bufs=2) as wp,
            tc.tile_pool(name="xpool", bufs=2) as xp,
            tc.tile_pool(name="hpool", bufs=2) as hp,
            tc.tile_pool(name="ypool", bufs=4) as yp,
            tc.tile_pool(name="bias", bufs=1) as bp,
            tc.tile_pool(name="psh", bufs=4, space="PSUM") as psh,
            tc.tile_pool(name="psy", bufs=3, space="PSUM") as psy,
        ):
            w1_sbs = [None] * E_LOCAL
            w2_sbs = [None] * E_LOCAL
            xt_sbs = [None] * E_LOCAL

            def load_slot(e, split_first):
                xt_sb = xp.tile([128, KD, CAPMAX], bf16, tag="xt")
                if split_first:
                    # per-k DMAs so the first matmul starts after ~0.3 MB
                    for k in range(KD):
                        nc.sync.dma_start(
                            out=xt_sb[:, k, :caps[e]],
                            in_=xt_d[k, :, xoff[e]:xoff[e + 1]],
                        )
                else:
                    nc.sync.dma_start(
                        out=xt_sb[:, :, :caps[e]],
                        in_=xt_d[:, :, xoff[e]:xoff[e + 1]].rearrange(
                            "k p t -> p k t"
                        ),
                    )
                w1_sb = wp.tile([128, KD, F], bf16, tag="w1")
                if split_first:
                    # first f-tile alone, then the rest: PE starts early
                    for f0, f1 in ((0, 128), (128, 1024), (1024, F)):
                        nc.sync.dma_start(
                            out=w1_sb[:, :, f0:f1],
                            in_=w1_d[e, :, :, f0:f1].rearrange("k p f -> p k f"),
                        )
                else:
                    nc.sync.dma_start(
                        out=w1_sb[:], in_=w1_d[e].rearrange("k p f -> p k f")
                    )
                w2_sb = wp.tile([128, KF, D], bf16, tag="w2")
                nc.sync.dma_start(
                    out=w2_sb[:], in_=w2_d[e].rearrange("k p f -> p k f")
                )
                xt_sbs[e], w1_sbs[e], w2_sbs[e] = xt_sb, w1_sb, w2_sb

            # slot 0 inputs issued first so PE can start ASAP
            load_slot(0, split_first=True)

            # biases (small / off critical path; b2 broadcast on gpsimd queue)
            b1_sb = bp.tile([128, E_LOCAL, KF], fp32)
            nc.gpsimd.dma_start(out=b1_sb[:], in_=b1_d[:].rearrange("e p f -> p e f"))
            b2_sb = bp.tile([128, E_LOCAL, D], fp32)
            b2_ap = b2_d[:]
            b2_bc = bass.AP(
                tensor=b2_ap.tensor,
                offset=b2_ap.offset,
                ap=[[0, 128]] + [list(a) for a in b2_ap.ap],
            )
            nc.gpsimd.dma_start(out=b2_sb[:], in_=b2_bc)

            for e in range(E_LOCAL):
                cap = caps[e]
                if e + 1 < E_LOCAL:
                    load_slot(e + 1, split_first=False)
                w1_sb, w2_sb, xt_sb = w1_sbs[e], w2_sbs[e], xt_sbs[e]

                # layer-1 token chunks (PSUM free dim <= 512 fp32).
                # Balanced halves for cap > 512: a tiny second chunk would
                # pay a full LDWEIGHTS per matmul for a handful of columns.
                if cap <= 512:
                    chunks = [(0, cap)]
                else:
                    h = (cap + 1) // 2
                    chunks = [(0, h), (h, cap - h)]

                # layer 1: HT[f-tile, tok] = gelu(w1_tile.T @ XT + b1)
                ht_sb = hp.tile([128, KF, CAPMAX], bf16, tag="ht")
                for f in range(KF):
                    for (c0, cs) in chunks:
                        ph = psh.tile([128, 512], fp32, tag="ph")
                        for k in range(KD):
                            nc.tensor.matmul(
                                ph[:, :cs],
                                lhsT=w1_sb[:, k, f * 128:(f + 1) * 128],
                                rhs=xt_sb[:, k, c0:c0 + cs],
                                start=(k == 0),
                                stop=(k == KD - 1),
                            )
                        nc.scalar.activation(
                            out=ht_sb[:, f, c0:c0 + cs],
                            in_=ph[:, :cs],
                            func=mybir.ActivationFunctionType.Gelu,
                            bias=b1_sb[:, e, f:f + 1],
                            scale=1.0,
                        )

                # layer 2: Y[t-tile, :] = HT_tile.T @ w2 + b2
                NT = -(-cap // 128)
                for t in range(NT):
                    tt = min(128, cap - t * 128)
                    py = psy.tile([128, D], fp32, tag="py")
                    for k in range(KF):
                        nc.tensor.matmul(
                            py[:tt, :],
                            lhsT=ht_sb[:, k, t * 128:t * 128 + tt],
                            rhs=w2_sb[:, k, :],
                            start=(k == 0),
                            stop=(k == KF - 1),
                        )
                    y_sb = yp.tile([128, D], fp32, tag="ysb")
                    nc.vector.tensor_add(y_sb[:tt, :], py[:tt, :], b2_sb[:tt, e, :])
                    nc.sync.dma_start(
                        out=y_d[yoff[e] + t * 128: yoff[e] + t * 128 + tt, :],
                        in_=y_sb[:tt, :],
                    )

    nc.compile()
    return nc


def _get_nc(caps):
    key = tuple(caps)
    if key not in _nc_cache:
        _nc_cache[key] = _build_nc(key)
    return _nc_cache[key]


def kernel(**inputs):
    x = np.asarray(inputs["inputs"], dtype=np.float32)
    disp = np.asarray(inputs["dispatch_order"])
    w1 = np.asarray(inputs["w1"], dtype=np.float32)
    b1 = np.asarray(inputs["b1"], dtype=np.float32)
    w2 = np.asarray(inputs["w2"], dtype=np.float32)
    b2 = np.asarray(inputs["b2"], dtype=np.float32)

    B, S, Dd = x.shape
    assert Dd == D
    T = B * S
    xf = x.reshape(T, D)
    e = disp.astype(np.int64)

    counts = np.bincount(e, minlength=NUM_EXPERTS)
    order = np.argsort(e, kind="stable")
    xs = xf[order]  # tokens grouped by expert, original order within expert
    offs = np.zeros(NUM_EXPERTS + 1, dtype=np.int64)
    np.cumsum(counts, out=offs[1:])

    # assign experts to (slot, core): slot j of core c gets the (8j+c)-th
    # most-loaded expert -> tight per-slot caps, balanced cores
    by_load = np.argsort(-counts, kind="stable")
    slot_expert = by_load.reshape(E_LOCAL, N_CORES)  # [slot, core] -> expert id
    caps = tuple(int(counts[slot_expert[j]].max()) for j in range(E_LOCAL))
    xoff, yoff = _slot_geometry(caps)

    # weights in device layout
    w1b = w1.astype(_BF16).reshape(NUM_EXPERTS, KD, 128, F)
    w2b = w2.astype(_BF16).reshape(NUM_EXPERTS, KF, 128, D)
    b1r = np.ascontiguousarray(
        b1.reshape(NUM_EXPERTS, KF, 128).transpose(0, 2, 1)
    )  # [E, 128, KF]
    xsb = xs.astype(_BF16)

    in_maps = []
    for c in range(N_CORES):
        eids = [int(slot_expert[j, c]) for j in range(E_LOCAL)]
        xt = np.zeros((KD, 128, xoff[-1]), dtype=_BF16)
        for j, ei in enumerate(eids):
            cnt = int(counts[ei])
            if cnt:
                xe = xsb[offs[ei]:offs[ei + 1]]  # [cnt, D]
                xt[:, :, xoff[j]:xoff[j] + cnt] = xe.T.reshape(KD, 128, cnt)
        in_maps.append({
            "xt": xt,
            "w1": np.ascontiguousarray(w1b[eids]),
            "w2": np.ascontiguousarray(w2b[eids]),
            "b1": np.ascontiguousarray(b1r[eids]),
            "b2": np.ascontiguousarray(b2[eids]),
        })

    nc = _get_nc(caps)
    global _last_in_maps
    _last_in_maps = in_maps
    from concourse.bass_utils import run_bass_kernel_spmd

    res = run_bass_kernel_spmd(nc, in_maps, core_ids=list(range(N_CORES)))

    out_sorted = np.empty((T, D), dtype=np.float32)
    for c in range(N_CORES):
        y = res.results[c]["y"]
        for j in range(E_LOCAL):
            ei = int(slot_expert[j, c])
            cnt = int(counts[ei])
            if cnt:
                out_sorted[offs[ei]:offs[ei + 1]] = y[yoff[j]:yoff[j] + cnt]

    out = np.empty((T, D), dtype=np.float32)
    out[order] = out_sorted
    return out.reshape(B, S, D)
